# revision 3
# baseline (speedup 1.0000x reference)
"""Trainium2 Bass kernel for CapsNet dynamic routing (nn_CapsRoutingLayer).

Reference computation:
    x_hat[b,i,o,d] = sum_k W[i,o,d,k] * x[b,i,k]
    b_logits = 0
    for it in 0..2:
        c = softmax_o(b_logits); s[b,o,d] = sum_i c[b,i,o] x_hat[b,i,o,d]
        v = squash(s)   # global Frobenius norm over the whole s tensor
        if it < 2: b_logits += sum_d x_hat[b,i,o,d] v[b,o,d]
    return v  # (128, 32, 32)

Sharding: input capsules i (1152) split across 8 cores (144 each). Per-core:

  * All matmuls in bf16; W shard (9.4MB bf16) stays SBUF-resident, DMA'd in
    12 chunks so pass-0 matmuls start as soon as the first chunk lands.
  * Pass 0 (uniform c): s0 = (1/32) sum_i x_hat via K=128 matmuls that sum
    4 capsules per contraction.
  * Passes 1,2 regenerate x_hat per supergroup of 8 capsules into PSUM
    (2-capsule tiles, double-buffered); ACT casts it to bf16 SBUF; the bulk
    elementwise work (agreement mult, d-fold tree, c-weighted mult, capsule
    sum tree) runs on DVE in its 2x bf16 mode. HW measurement showed GpSimd
    runs ~1.8x SLOWER than its cost model while DVE runs faster, so - unlike
    the previous revision - the capsule-sum tree lives on DVE and GpSimd only
    gets the small ops (c=e*rz, z-reduce, the single per-supergroup s_acc
    accumulate, and the squash partition reduction).
  * Everything is in a (d,o)-transposed layout (untransposed on the host at
    the end) so both broadcast multiplies keep a packed innermost axis.
  * The two on-device AllReduces (s0, s1) run in bf16 (half the wire bytes);
    pass 1 uses the RAW s0 as its agreement multiplier with squash's g0
    folded into the softmax exp scale, so pass-1 DVE work starts right after
    the AllReduce. Pass 2 uses m2 = g1*s1 + v0 (logits are linear in v).
  * Pass 2 does NOT AllReduce or squash: each core emits its local partial
    s2; the host sums the 8 partials and applies the global-norm squash in
    numpy (exact fp32), killing the 3rd collective + device squash chain.
  * squash's partition reduction uses gpsimd.partition_all_reduce instead of
    PE matmul round-trips.

HW notes: nc.vector.tensor_tensor_reduce kills the worker on this setup;
scalar_tensor_tensor is fine but has no DVE fast modes (used only for the
tiny m2 update). Mixing 32-row tile_position matmuls with full-128
stationaries deep in the PE pipe corrupts results on HW (sim-clean), so the
capsule sum stays off the PE.
"""

import numpy as np
import ml_dtypes

from concourse import bacc, bass_isa, bass_utils, mybir, tile

N_CORES = 8
B = 128          # batch
NI = 1152        # input capsules
K = 32           # dim_input
NO = 32          # output capsules
D = 32           # dim_output
IC = NI // N_CORES   # input capsules per core = 144
NJ = IC // 4         # i-groups of 4 per core = 36
OD = NO * D          # 1024
SGC = 8              # capsules per supergroup
NSG = IC // SGC      # supergroups per pass = 18
NWC = 12             # W is DMA'd in NWC chunks of NJ//NWC j-groups
JC = NJ // NWC       # 3

F32 = mybir.dt.float32
BF16 = mybir.dt.bfloat16
ADD = mybir.AluOpType.add
MULT = mybir.AluOpType.mult
AXX = mybir.AxisListType.X
EXP = mybir.ActivationFunctionType.Exp

# Timing-ablation only: replace the cross-core AllReduce with a plain DMA
# (results become wrong; used to measure the collective's cost).
SKIP_COLLECTIVE = False


def _kernel_body(nc, tc, xs, ws, vout, repeats=1):
    with tc.tile_pool(name="persist", bufs=1) as per, \
         tc.tile_pool(name="xhp", bufs=4) as xhp, \
         tc.tile_pool(name="smallp", bufs=2) as smallp, \
         tc.tile_pool(name="pgp", bufs=2, space="PSUM") as pgp, \
         tc.tile_pool(name="dram", bufs=1, space="DRAM") as dram:

        W_t = [per.tile([128, JC, OD], BF16, name=f"W_t{c}") for c in range(NWC)]
        x_t = per.tile([128, NJ, 128], BF16)
        nc.sync.dma_start(x_t[:], xs[:])       # small; every matmul needs it
        for c in range(NWC):
            nc.sync.dma_start(W_t[c][:], ws[:, JC * c:JC * (c + 1), :])

        xv = per.tile([B, SGC * OD], BF16)     # agreement products
        sx = per.tile([B, SGC * OD], BF16)     # c-weighted x_hat
        s_acc = per.tile([B, OD], F32)         # local s accumulator
        s_bf = per.tile([B, OD], BF16)         # bf16 staging for AllReduce
        s_full = per.tile([B, OD], BF16)       # post-AllReduce s (= pass-1 m)
        v0 = per.tile([B, OD], BF16)           # squash(s0)
        m_bf = per.tile([B, OD], BF16)         # pass-2 agreement multiplier
        sq = per.tile([B, OD], BF16)           # squash scratch
        col = per.tile([B, 1], F32)
        Sb = per.tile([128, 1], F32)
        t1 = per.tile([128, 1], F32)
        t2 = per.tile([128, 1], F32)
        t3 = per.tile([128, 1], F32)
        gb = per.tile([128, 1], F32)

        ar_in = dram.tile([B, OD], BF16)
        ar_out = dram.tile([B, OD], BF16)

        def allreduce(src_bf):
            nc.sync.dma_start(ar_in[:], src_bf[:])
            if SKIP_COLLECTIVE:
                nc.sync.dma_start(ar_out[:], ar_in[:])
            else:
                nc.gpsimd.collective_compute(
                    "AllReduce", ADD,
                    replica_groups=[list(range(N_CORES))],
                    ins=[ar_in.opt()], outs=[ar_out.opt()],
                )
            nc.sync.dma_start(s_full[:], ar_out[:])

        def squash_mult(pass_idx):
            # g = sqrt(S)/(1+S) with S = global sum of squares of s_full.
            # Pass 1 uses the RAW s0 (= s_full) as its agreement multiplier
            # and folds g0 into the softmax exp's per-partition scale
            # (exp(g0*a) with a=<xh,s0> equals exp(<xh,v0>)).
            nc.vector.tensor_mul(sq[:], s_full[:], s_full[:])
            nc.vector.tensor_reduce(out=col[:], in_=sq[:], axis=AXX, op=ADD)
            nc.gpsimd.partition_all_reduce(Sb[:], col[:], channels=128,
                                           reduce_op=bass_isa.ReduceOp.add)
            nc.scalar.sqrt(t1[:], Sb[:])
            nc.vector.tensor_scalar_add(t2[:], Sb[:], 1.0)
            nc.vector.reciprocal(t3[:], t2[:])
            nc.vector.tensor_mul(gb[:], t1[:], t3[:])
            if pass_idx == 0:
                nc.vector.tensor_scalar_mul(v0[:], s_full[:], gb[:])
            else:
                nc.vector.scalar_tensor_tensor(             # m2 = g1*s1+v0
                    out=m_bf[:], in0=s_full[:], scalar=gb[:], in1=v0[:],
                    op0=MULT, op1=ADD)

        def regen_supergroup(S):
            # PE: x_hat for capsules 8S..8S+7 -> PSUM; ACT: cast-copy to SBUF
            xh = xhp.tile([B, SGC * OD], BF16, name="xh", tag="xh")
            for g4 in range(SGC // 2):
                pg = pgp.tile([B, 2 * OD], F32, name="pg", tag="pg")
                for slot in range(2):
                    i = SGC * S + 2 * g4 + slot
                    j, gg = divmod(i, 4)
                    wt = W_t[j // JC]
                    jj = j % JC
                    for h in range(2):
                        lo = slot * OD + 512 * h
                        nc.tensor.matmul(
                            pg[:, lo:lo + 512],
                            x_t[32 * gg:32 * (gg + 1), j, :],
                            wt[32 * gg:32 * (gg + 1), jj, 512 * h:512 * (h + 1)],
                            start=True, stop=True, tile_position=(32 * gg, 0))
                nc.scalar.copy(xh[:, 2 * OD * g4:2 * OD * (g4 + 1)], pg[:])
            return xh

        def run_pass(r):
            m = s_full if r == 1 else m_bf
            state = {}
            for S in range(NSG + 1):
                if S < NSG:
                    xh = regen_supergroup(S)
                    # agreement multiply: xv = xh * m (broadcast over capsule)
                    nc.vector.tensor_tensor(
                        out=xv[:].rearrange("b (i f) -> b i f", i=SGC),
                        in0=xh[:].rearrange("b (i f) -> b i f", i=SGC),
                        in1=m[:].unsqueeze(1).broadcast_to([B, SGC, OD]),
                        op=MULT)
                if S >= 1:
                    st = state[S - 1]
                    nc.vector.tensor_reduce(
                        out=st["z"][:],
                        in_=st["e"][:].rearrange("b (i o) -> b i o", i=SGC),
                        axis=AXX, op=ADD)
                    nc.vector.reciprocal(st["rz"][:], st["z"][:])
                if S < NSG:
                    # reduce over d (the MIDDLE axis in the [b,i,d,o] layout,
                    # so every fold keeps a packed o innermost and runs in the
                    # DVE 2x mode) as a log2 fold tree of adds.
                    a16 = smallp.tile([B, SGC * NO], BF16, name="a16", tag="a")
                    v4 = xv[:].rearrange("b (i d o) -> b i d o", i=SGC, d=D)
                    w = D // 2
                    while w > 1:
                        nc.vector.tensor_add(v4[:, :, 0:w, :], v4[:, :, 0:w, :],
                                             v4[:, :, w:2 * w, :])
                        w //= 2
                    nc.vector.tensor_add(
                        a16[:].rearrange("b (i o) -> b i o", i=SGC)
                            .unsqueeze(2),
                        v4[:, :, 0:1, :], v4[:, :, 1:2, :])
                    e16 = smallp.tile([B, SGC * NO], BF16, name="e16", tag="e")
                    nc.scalar.activation(e16[:], a16[:], EXP,
                                         scale=gb[:] if r == 1 else 1.0)
                    z16 = smallp.tile([B, SGC], F32, name="z16", tag="z")
                    rz16 = smallp.tile([B, SGC], F32, name="rz16", tag="rz")
                    state[S] = dict(e=e16, z=z16, rz=rz16, xh=xh)
                if S >= 1:
                    st = state[S - 1]
                    c16 = smallp.tile([B, SGC * NO], BF16, name="c16", tag="c")
                    nc.gpsimd.tensor_tensor(
                        out=c16[:].rearrange("b (i o) -> b i o", i=SGC),
                        in0=st["e"][:].rearrange("b (i o) -> b i o", i=SGC),
                        in1=st["rz"][:].unsqueeze(2).broadcast_to([B, SGC, NO]),
                        op=MULT)
                    # s contribution: sx = xh * c (broadcast over d; d is the
                    # middle axis so the innermost o stays packed -> DVE 2x)
                    nc.vector.tensor_tensor(
                        out=sx[:].rearrange("b (i d o) -> b i d o", i=SGC, d=D),
                        in0=st["xh"][:].rearrange("b (i d o) -> b i d o",
                                                  i=SGC, d=D),
                        in1=c16[:].rearrange("b (i o) -> b i o", i=SGC)
                            .unsqueeze(2).broadcast_to([B, SGC, D, NO]),
                        op=MULT)
                    # capsule-sum tree, all on DVE (HW GpSimd is ~7x slower
                    # per element than DVE's bf16 2x mode); GpSimd only does
                    # the single serial fp32 accumulate into s_acc.
                    nc.vector.tensor_add(sx[:, 0:OD], sx[:, 0:OD],
                                         sx[:, OD:2 * OD])
                    nc.vector.tensor_add(sx[:, 2 * OD:3 * OD],
                                         sx[:, 2 * OD:3 * OD],
                                         sx[:, 3 * OD:4 * OD])
                    nc.vector.tensor_add(sx[:, 4 * OD:5 * OD],
                                         sx[:, 4 * OD:5 * OD],
                                         sx[:, 5 * OD:6 * OD])
                    nc.vector.tensor_add(sx[:, 6 * OD:7 * OD],
                                         sx[:, 6 * OD:7 * OD],
                                         sx[:, 7 * OD:8 * OD])
                    nc.vector.tensor_add(sx[:, 0:OD], sx[:, 0:OD],
                                         sx[:, 2 * OD:3 * OD])
                    nc.vector.tensor_add(sx[:, 4 * OD:5 * OD],
                                         sx[:, 4 * OD:5 * OD],
                                         sx[:, 6 * OD:7 * OD])
                    nc.vector.tensor_add(sx[:, 0:OD], sx[:, 0:OD],
                                         sx[:, 4 * OD:5 * OD])
                    if S - 1 == 0:
                        nc.gpsimd.tensor_copy(s_acc[:], sx[:, 0:OD])
                    else:
                        nc.gpsimd.tensor_add(s_acc[:], s_acc[:], sx[:, 0:OD])

        with nc.allow_low_precision("bf16 routing; tolerance is 2e-2"):
            for _rep in range(repeats):
                # ---- pass 0: s0 = (1/32) sum_i x_hat, K=128 matmuls
                pg0 = pgp.tile([B, 2 * OD], F32, name="pg0", tag="pg")
                for j in range(NJ):
                    wt = W_t[j // JC]
                    jj = j % JC
                    for h in range(2):
                        nc.tensor.matmul(
                            pg0[:, 512 * h:512 * (h + 1)],
                            x_t[:, j, :], wt[:, jj, 512 * h:512 * (h + 1)],
                            start=(j == 0), stop=(j == NJ - 1))
                nc.vector.tensor_scalar_mul(s_bf[:], pg0[:, 0:OD], 1.0 / NO)
                allreduce(s_bf)
                squash_mult(0)
                run_pass(1)
                nc.vector.tensor_copy(s_bf[:], s_acc[:])
                allreduce(s_bf)
                squash_mult(1)
                run_pass(2)
                # pass 2: emit the local partial s2; the host sums the 8
                # partials and applies the global-norm squash in fp32.
                nc.sync.dma_start(vout[:], s_acc[:])


_NC_CACHE = {}


def _build(repeats=1):
    if repeats in _NC_CACHE:
        return _NC_CACHE[repeats]
    nc = bacc.Bacc("TRN2", target_bir_lowering=False, debug=False,
                   num_devices=N_CORES)
    xs = nc.dram_tensor("xs", [128, NJ, 128], BF16, kind="ExternalInput").ap()
    ws = nc.dram_tensor("ws", [128, NJ, OD], BF16, kind="ExternalInput").ap()
    vout = nc.dram_tensor("v", [B, OD], F32, kind="ExternalOutput").ap()
    with tile.TileContext(nc) as tc:
        _kernel_body(nc, tc, xs, ws, vout, repeats=repeats)
    nc.compile()
    _NC_CACHE[repeats] = nc
    return nc


def _shard_inputs(x, W):
    BF = ml_dtypes.bfloat16
    in_maps = []
    for c in range(N_CORES):
        i0 = c * IC
        wc = W[i0:i0 + IC]                          # (144, 32, 32, 32) iodk
        # (d,o)-transposed columns: ws[(g,k), j, (d,o)] = W[i0+4j+g, o, d, k]
        wsn = np.ascontiguousarray(
            wc.reshape(NJ, 4, NO, D, K).transpose(1, 4, 0, 3, 2)
              .reshape(128, NJ, OD)).astype(BF)
        xc = x[:, i0:i0 + IC, :]                    # (128, 144, 32) bik
        xt = np.ascontiguousarray(
            xc.reshape(B, NJ, 4, K).transpose(2, 3, 1, 0)
              .reshape(128, NJ, 128)).astype(BF)
        in_maps.append({"xs": xt, "ws": wsn})
    return in_maps


def kernel(x, W, _trace=False):
    x = np.asarray(x, dtype=np.float32)
    W = np.asarray(W, dtype=np.float32)
    nc = _build()
    in_maps = _shard_inputs(x, W)
    res = bass_utils.run_bass_kernel_spmd(
        nc, in_maps, core_ids=list(range(N_CORES)), trace=_trace)
    # Each core returns its local partial of s2 (sum over its capsule shard);
    # finish routing iteration 3 on the host: global sum + squash, then
    # untranspose the (d,o) layout.
    s2 = np.zeros((B, OD), np.float64)
    for c in range(N_CORES):
        s2 += res.results[c]["v"].astype(np.float64)
    n = np.linalg.norm(s2)
    v = (n / (1.0 + n * n)) * s2
    out = np.ascontiguousarray(
        v.reshape(B, D, NO).transpose(0, 2, 1)).astype(np.float32)
    if _trace:
        kernel.last_exec_time_ns = res.exec_time_ns
        kernel.last_results = res
    return out
